# revision 3
# baseline (speedup 1.0000x reference)
"""GQA kernel for Trainium2: B=2,T=2048,E=2048,G=4,QPG=4,D=128, causal + sinusoidal PE.

Sharding: one core per (batch, kv-group) pair = 2*4 = 8 cores.
Each core computes q/k/v projections for its group, attention for its 4 query
heads, and a partial output projection (its group's 512 columns of wo);
partials are summed on the host (f32 accumulate of bf16 partials).

Layout strategy (per core):
  - all matmul streams are bf16 (halves DMA traffic vs f32, enables FWL
    ~54ns LDWEIGHTS everywhere); PSUM accumulation stays f32.
  - weights stream per-e-tile on the scalar DGE ring interleaved in
    consumption order; x^T streams on the sync ring (per-e for t-block 0,
    then [128,2048] batched tiles); constants + wo on the gpsimd ring.
  - scores are computed transposed: S^T[tk, tq] = kt_tile.T @ qt, softmax'd
    without max subtraction (scores bounded, verified), exp'd into bf16 P^T
    tiles that feed the PV matmul directly as lhsT. Diagonal tiles only
    compute the valid suffix columns. qb=0's scores are emitted inside
    phase 1 (they only need t-block 0) to fill DMA-stall gaps and eliminate
    the phase transition bubble.
  - denominator comes free via a ones-column appended to V (N=129).
  - attention output [tq, d] is normalized via per-partition scale (bf16),
    then PE-transposed to [d, tq] to feed the wo matmul as lhsT.
"""
import sys

sys.path.insert(0, "/opt/trn_rl_repo")

import math
import numpy as np

B, T, E = 2, 2048, 2048
G, QPG, D = 4, 4, 128
NQ = QPG * D          # 512 q columns per group
NKV = 2 * D           # 256 kv columns per group
TT = T // 128         # 16 t-tiles
TB = T // 512         # 4 t-blocks
NE = E // 128         # 16 e-tiles
ISD = 1.0 / math.sqrt(D)

_compiled = None


def _build():
    from concourse import bacc, tile, mybir

    f32 = mybir.dt.float32
    bf16 = mybir.dt.bfloat16
    ADD = mybir.AluOpType.add
    MULT = mybir.AluOpType.mult
    EXP = mybir.ActivationFunctionType.Exp
    IDENT = mybir.ActivationFunctionType.Identity

    nc = bacc.Bacc("TRN2", target_bir_lowering=False, debug=False, num_devices=8)

    xt_d = nc.dram_tensor("xt", [TB, 4, 128, 4 * 512], bf16, kind="ExternalInput")   # x^T blocked
    wq_d = nc.dram_tensor("wq", [E, NQ], bf16, kind="ExternalInput")                 # group slice
    wkv_d = nc.dram_tensor("wkv", [E, NKV], bf16, kind="ExternalInput")              # group slice
    wo_d = nc.dram_tensor("wo", [4, 128, E], bf16, kind="ExternalInput")             # group slice
    pet_d = nc.dram_tensor("pet", [D, T], bf16, kind="ExternalInput")                # pe^T
    bq_d = nc.dram_tensor("bq", [D, QPG], f32, kind="ExternalInput")                 # col h
    bk_d = nc.dram_tensor("bk", [D, 1], f32, kind="ExternalInput")
    bv_d = nc.dram_tensor("bv", [D, 1], f32, kind="ExternalInput")
    msk_d = nc.dram_tensor("msk", [4, 128, 512], bf16, kind="ExternalInput")
    idb_d = nc.dram_tensor("idb", [128, 128], bf16, kind="ExternalInput")
    ones_d = nc.dram_tensor("ones1", [128, 1], bf16, kind="ExternalInput")
    out_d = nc.dram_tensor("out", [T, E], bf16, kind="ExternalOutput")

    with tile.TileContext(nc) as tc:
        with tc.tile_pool(name="persist", bufs=1) as pp, \
             tc.tile_pool(name="p2", bufs=20) as p2:
            qt = [pp.tile([128, T], bf16, name=f"qt{h}", tag=f"qt{h}") for h in range(QPG)]
            kt = pp.tile([128, T], bf16)
            vext = [pp.tile([128, 132], bf16, name=f"vx{i}", tag=f"vx{i}") for i in range(TT)]
            at = [pp.tile([128, T], bf16, name=f"at{h}", tag=f"at{h}") for h in range(QPG)]
            wo_sb = [pp.tile([128, E], bf16, name=f"wo{h}", tag=f"wo{h}") for h in range(QPG)]

            pet = pp.tile([D, T], bf16)
            bq = pp.tile([D, QPG], f32)
            bk = pp.tile([D, 1], f32)
            bv = pp.tile([D, 1], f32)
            idb = pp.tile([128, 128], bf16)
            ones1 = pp.tile([128, 1], bf16)
            msk = [pp.tile([128, 512], bf16, name=f"msk{j}", tag=f"msk{j}") for j in range(4)]

            pt_q0 = {}   # h -> [4 P^T tiles], scores computed inside phase 1

            # ---- phase 1: projections (+ qb0 attention scores as filler) ----
            with (
                tc.tile_pool(name="p1", bufs=1) as p1,
                tc.tile_pool(name="p1x", bufs=8) as p1x,
                tc.tile_pool(name="p1e", bufs=16) as p1e,
                tc.tile_pool(name="psA", bufs=1, space="PSUM") as psA,
                tc.tile_pool(name="ps1b", bufs=1, space="PSUM") as ps1b,
                tc.tile_pool(name="sA", bufs=1, space="PSUM") as sA,
            ):
                wq_sb = [p1.tile([128, NQ], bf16, name=f"wq{e}", tag=f"wq{e}") for e in range(NE)]
                wkv_sb = [p1.tile([128, NKV], bf16, name=f"wkv{e}", tag=f"wkv{e}") for e in range(NE)]
                xt0 = [p1e.tile([128, 512], bf16, name=f"xt0_{e}", tag="xt0") for e in range(NE)]
                # consumption-ordered fine-grained loads: scalar ring carries
                # weights, sync ring carries xt; first e-iteration's operands
                # are the first triggers on each ring.
                for e in range(NE):
                    a, b = divmod(e, 4)
                    nc.scalar.dma_start(wq_sb[e][:], wq_d[e * 128:(e + 1) * 128, :])
                    nc.scalar.dma_start(wkv_sb[e][:], wkv_d[e * 128:(e + 1) * 128, :])
                    nc.sync.dma_start(xt0[e][:], xt_d[0, a, :, b * 512:(b + 1) * 512])
                # constants on gpsimd ring, in first-use order
                nc.gpsimd.dma_start(bq[:], bq_d[:])
                nc.gpsimd.dma_start(bk[:], bk_d[:])
                nc.gpsimd.dma_start(bv[:], bv_d[:])
                nc.gpsimd.dma_start(idb[:], idb_d[:])
                nc.gpsimd.dma_start(ones1[:], ones_d[:])
                for j in range(4):
                    nc.gpsimd.dma_start(msk[j][:], msk_d[j])
                nc.gpsimd.dma_start(pet[:], pet_d[:])

                def q0_scores(h):
                    # qb0 score stretch for head h: 4 diagonal tiles, suffix only
                    pt = []
                    for tk in range(4):
                        off = tk * 128
                        w = 512 - off
                        s_ps = sA.tile([128, 512], f32, name="s0", tag="s0")
                        nc.tensor.matmul(
                            s_ps[:, 0:w], kt[:, tk * 128:(tk + 1) * 128],
                            qt[h][:, off:512], start=True, stop=True,
                        )
                        p_t = p2.tile([128, 512], bf16, name="pt", tag="pt")
                        nc.scalar.activation(p_t[:, off:], s_ps[:, 0:w], EXP, scale=ISD)
                        nc.vector.tensor_tensor(p_t[:, off:], p_t[:, off:], msk[tk][:, off:], MULT)
                        pt.append(p_t)
                    pt_q0[h] = pt

                for tb in range(TB):
                    ts = slice(tb * 512, (tb + 1) * 512)
                    qt_ps = psA.tile([128, 4 * 512], f32, name="qt_ps", tag="qt_ps")
                    kt_ps = psA.tile([128, 512], f32, name="kt_ps", tag="kt_ps")
                    vt_ps = psA.tile([128, 512], f32, name="vt_ps", tag="vt_ps")
                    for e in range(NE):
                        a, b = divmod(e, 4)
                        if tb == 0:
                            xt_t = xt0[e]
                            xs = slice(0, 512)
                        else:
                            if b == 0:
                                xt_t = p1x.tile([128, 4 * 512], bf16, name="xt", tag="xt")
                                nc.sync.dma_start(xt_t[:], xt_d[tb, a])
                            xs = slice(b * 512, (b + 1) * 512)
                        st = e == 0
                        sp = e == NE - 1
                        for h in range(QPG):
                            nc.tensor.matmul(
                                qt_ps[:, h * 512:(h + 1) * 512],
                                wq_sb[e][:, h * 128:(h + 1) * 128],
                                xt_t[:, xs], start=st, stop=sp,
                            )
                        nc.tensor.matmul(kt_ps[:], wkv_sb[e][:, 0:128], xt_t[:, xs], start=st, stop=sp)
                        nc.tensor.matmul(vt_ps[:], wkv_sb[e][:, 128:256], xt_t[:, xs], start=st, stop=sp)
                        # interleave qb0 attention score stretches as DMA-stall filler
                        if tb == 1 and e == 7:
                            q0_scores(0)
                        elif tb == 1 and e == 15:
                            q0_scores(1)
                        elif tb == 2 and e == 7:
                            q0_scores(2)
                        elif tb == 2 and e == 15:
                            q0_scores(3)
                    # drain: bias (in-place on psum) then += pe^T -> sbuf bf16
                    for h in range(QPG):
                        sl = qt_ps[:, h * 512:(h + 1) * 512]
                        nc.vector.tensor_tensor(sl, sl, bq[:, h:h + 1].to_broadcast([128, 512]), ADD)
                        nc.vector.tensor_tensor(qt[h][:, ts], sl, pet[:, ts], ADD)
                    nc.vector.tensor_tensor(kt_ps[:], kt_ps[:], bk[:].to_broadcast([128, 512]), ADD)
                    nc.vector.tensor_tensor(kt[:, ts], kt_ps[:], pet[:, ts], ADD)
                    # v: bias then cast to bf16, then transpose each 128-tile
                    vtb = p1.tile([128, 512], bf16, name="vtb", tag="vtb")
                    nc.scalar.activation(vtb[:], vt_ps[:], IDENT, bias=bv[:], scale=1.0)
                    for i in range(4):
                        ti = tb * 4 + i
                        vtp = ps1b.tile([128, 128], bf16, name="vtp", tag="vtp")
                        nc.tensor.transpose(vtp[:], vtb[:, i * 128:(i + 1) * 128], idb[:])
                        nc.vector.tensor_copy(vext[ti][:, 0:128], vtp[:])
                        nc.vector.tensor_copy(vext[ti][:, 128:129], ones1[:])

            for h in range(QPG):
                nc.gpsimd.dma_start(wo_sb[h][:], wo_d[h])

            # ---- phase 2+3: attention fused with output projection ----
            with (
                tc.tile_pool(name="p2s", bufs=8) as p2s,
                tc.tile_pool(name="p3", bufs=3) as p3,
                tc.tile_pool(name="ps2", bufs=2, space="PSUM") as ps2,
            ):
                from collections import deque
                filler = deque()

                def drain(n):
                    for _ in range(n):
                        if not filler:
                            return
                        filler.popleft()()

                def wo_units(qb):
                    units = []
                    for jj in range(4):
                        ti = qb * 4 + jj
                        state = {}

                        def alloc(state=state):
                            state["o_sb"] = p3.tile([128, E], bf16, name="osb", tag="osb")
                        units.append(alloc)
                        for eo in range(4):
                            def mmA(state=state, ti=ti, eo=eo):
                                w_ps = ps2.tile([128, 512], f32, name="w_ps", tag="mix", bufs=2)
                                state["w"] = w_ps
                                for h in range(2):
                                    nc.tensor.matmul(
                                        w_ps[:], at[h][:, ti * 128:(ti + 1) * 128],
                                        wo_sb[h][:, eo * 512:(eo + 1) * 512],
                                        start=(h == 0), stop=False,
                                    )

                            def mmB(state=state, ti=ti, eo=eo):
                                w_ps = state["w"]
                                for h in range(2, 4):
                                    nc.tensor.matmul(
                                        w_ps[:], at[h][:, ti * 128:(ti + 1) * 128],
                                        wo_sb[h][:, eo * 512:(eo + 1) * 512],
                                        start=False, stop=(h == 3),
                                    )
                                nc.vector.tensor_copy(state["o_sb"][:, eo * 512:(eo + 1) * 512], w_ps[:])
                                nc.sync.dma_start(
                                    out_d[ti * 128:(ti + 1) * 128, eo * 512:(eo + 1) * 512],
                                    state["o_sb"][:, eo * 512:(eo + 1) * 512],
                                )
                            units.append(mmA)
                            units.append(mmB)
                    return units

                def make_pv_streak(h, qb, pt, and_then=None):
                    def emit():
                        # pure bf16 PV streak: all four tq sub-tiles back to back
                        o_list = []
                        for j in range(4):
                            tt = 4 * qb + j
                            o_ps = ps2.tile([128, 129], f32, name="o_ps", tag="o_ps", bufs=4)
                            o_list.append(o_ps)
                            for tk in range(tt + 1):
                                nc.tensor.matmul(
                                    o_ps[:], pt[tk][:, j * 128:(j + 1) * 128],
                                    vext[tk][:, 0:129],
                                    start=(tk == 0), stop=(tk == tt),
                                )
                        for j in range(4):
                            tt = 4 * qb + j
                            o_ps = o_list[j]
                            r_sb = p2s.tile([128, 1], f32, name="r", tag="r")
                            nc.vector.reciprocal(r_sb[:], o_ps[:, 128:129])
                            a_sb = p2s.tile([128, 128], bf16, name="a", tag="a")
                            nc.vector.tensor_tensor(
                                a_sb[:], o_ps[:, 0:128], r_sb[:].to_broadcast([128, 128]), MULT,
                            )
                            at_ps = ps2.tile([128, 128], bf16, name="at_ps", tag="mix", bufs=2)
                            nc.tensor.transpose(at_ps[:], a_sb[:], idb[:])
                            nc.vector.tensor_copy(at[h][:, tt * 128:(tt + 1) * 128], at_ps[:])
                        if and_then is not None:
                            and_then()
                    return emit

                pending_pv = None
                for qb in range(TB):
                    nkt = 4 * qb + 4
                    for h in range(QPG):
                        if qb == 0:
                            if pending_pv is not None:
                                pending_pv()
                                pending_pv = None
                            pt = pt_q0[h]
                        else:
                            pt = []
                            for tk in range(nkt):
                                j = tk - 4 * qb
                                off = max(j, 0) * 128
                                w = 512 - off
                                qcols = slice(qb * 512 + off, (qb + 1) * 512)
                                s_ps = ps2.tile([128, 512], f32, name="s_ps", tag="s_ps")
                                nc.tensor.matmul(
                                    s_ps[:, 0:w], kt[:, tk * 128:(tk + 1) * 128], qt[h][:, qcols],
                                    start=True, stop=True,
                                )
                                p_t = p2.tile([128, 512], bf16, name="pt", tag="pt")
                                nc.scalar.activation(p_t[:, off:], s_ps[:, 0:w], EXP, scale=ISD)
                                if j >= 0:
                                    nc.vector.tensor_tensor(p_t[:, off:], p_t[:, off:], msk[j][:, off:], MULT)
                                pt.append(p_t)
                                if tk == 1 and pending_pv is not None:
                                    pending_pv()
                                    pending_pv = None
                                else:
                                    drain(1)
                        cb = None
                        if h == QPG - 1:
                            def cb(qb=qb):
                                filler.extend(wo_units(qb))
                        pending_pv = make_pv_streak(h, qb, pt, and_then=cb)
                if pending_pv is not None:
                    pending_pv()
                drain(len(filler) + 1)

    nc.compile()
    return nc


def _get_compiled():
    global _compiled
    if _compiled is None:
        _compiled = _build()
    return _compiled


def _host_inputs(x, wq, bq, wkv, bkv, wo):
    import ml_dtypes

    bf = ml_dtypes.bfloat16

    pos = np.arange(T, dtype=np.float32)[:, None]
    i = np.arange(0, D, 2, dtype=np.float32)
    inv = np.exp(-(np.log(10000.0) * i / D))
    ang = pos * inv
    pe = np.zeros((T, D), np.float32)
    pe[:, 0::2] = np.sin(ang)
    pe[:, 1::2] = np.cos(ang)
    pet = np.ascontiguousarray(pe.T).astype(bf)

    # causal masks for the 4 diagonal tiles of a 512-wide tq block:
    # mask_j[p, c] = 1 if c >= 128*j + p
    c = np.arange(512)[None, :]
    p = np.arange(128)[:, None]
    msk = np.stack([(c >= 128 * j + p) for j in range(4)]).astype(bf)

    idb = np.eye(128, dtype=bf)
    ones1 = np.ones((128, 1), dtype=bf)

    # x^T blocked: [tb, a, p, b*512+c] = xT[(4a+b)*128+p, tb*512+c]
    xts = []
    for b_ in range(B):
        xT = np.ascontiguousarray(x[b_].T).astype(bf)          # [E, T]
        xb = xT.reshape(4, 4, 128, 4, 512).transpose(3, 0, 2, 1, 4).reshape(TB, 4, 128, 4 * 512)
        xts.append(np.ascontiguousarray(xb))

    in_maps = []
    for core in range(8):
        b_, g = divmod(core, G)
        wog = wo[g * NQ:(g + 1) * NQ, :].astype(bf).reshape(4, 128, E)
        in_maps.append({
            "xt": xts[b_],
            "wq": np.ascontiguousarray(wq[:, g * NQ:(g + 1) * NQ].astype(bf)),
            "wkv": np.ascontiguousarray(wkv[:, g * NKV:(g + 1) * NKV].astype(bf)),
            "wo": np.ascontiguousarray(wog),
            "pet": pet,
            "bq": np.ascontiguousarray(bq[g * NQ:(g + 1) * NQ].reshape(QPG, D).T),
            "bk": np.ascontiguousarray(bkv[g * NKV:g * NKV + D].reshape(D, 1)),
            "bv": np.ascontiguousarray(bkv[g * NKV + D:(g + 1) * NKV].reshape(D, 1)),
            "msk": msk,
            "idb": idb,
            "ones1": ones1,
        })
    return in_maps


def run(x, wq, bq, wkv, bkv, wo, trace=False):
    from concourse.bass_utils import run_bass_kernel_spmd

    nc = _get_compiled()
    in_maps = _host_inputs(
        np.asarray(x, np.float32), np.asarray(wq, np.float32),
        np.asarray(bq, np.float32), np.asarray(wkv, np.float32),
        np.asarray(bkv, np.float32), np.asarray(wo, np.float32),
    )
    res = run_bass_kernel_spmd(nc, in_maps, core_ids=list(range(8)), trace=trace)
    out = np.zeros((B, T, E), np.float32)
    for core in range(8):
        b_ = core // G
        out[b_] += res.results[core]["out"].astype(np.float32)
    return out, res


def kernel(x, wq, bq, wkv, bkv, wo):
    out, _ = run(x, wq, bq, wkv, bkv, wo, trace=False)
    return out


# revision 7
# speedup vs baseline: 1.0267x; 1.0267x over previous
"""GQA kernel for Trainium2: B=2,T=2048,E=2048,G=4,QPG=4,D=128, causal + sinusoidal PE.

Sharding: one core per (batch, kv-group) pair = 2*4 = 8 cores.
Each core computes q/k/v projections for its group, attention for its 4 query
heads, and a partial output projection (its group's 512 columns of wo);
partials are summed on the host (f32 accumulate of bf16 partials).

Layout strategy (per core):
  - all matmul streams are bf16 (halves DMA traffic vs f32, enables FWL
    ~54ns LDWEIGHTS everywhere); PSUM accumulation stays f32.
  - weights stream per-e-tile on the scalar DGE ring interleaved in
    consumption order; x^T streams on the sync ring (per-e for t-block 0,
    then [128,2048] batched tiles); constants + wo on the gpsimd ring.
  - scores are computed transposed: S^T[tk, tq] = kt_tile.T @ qt, softmax'd
    without max subtraction (scores bounded, verified), exp'd into bf16 P^T
    tiles that feed the PV matmul directly as lhsT. Diagonal tiles only
    compute the valid suffix columns. qb=0's scores are emitted inside
    phase 1 (they only need t-block 0) to fill DMA-stall gaps and eliminate
    the phase transition bubble.
  - denominator comes free via a ones-column appended to V (N=129).
  - attention output [tq, d] is normalized via per-partition scale (bf16),
    then PE-transposed to [d, tq] to feed the wo matmul as lhsT.
"""
import sys

sys.path.insert(0, "/opt/trn_rl_repo")

import math
import numpy as np

B, T, E = 2, 2048, 2048
G, QPG, D = 4, 4, 128
NQ = QPG * D          # 512 q columns per group
NKV = 2 * D           # 256 kv columns per group
TT = T // 128         # 16 t-tiles
TB = T // 512         # 4 t-blocks
NE = E // 128         # 16 e-tiles
ISD = 1.0 / math.sqrt(D)

_compiled = None


def _build():
    from concourse import bacc, tile, mybir

    f32 = mybir.dt.float32
    bf16 = mybir.dt.bfloat16
    ADD = mybir.AluOpType.add
    MULT = mybir.AluOpType.mult
    EXP = mybir.ActivationFunctionType.Exp
    IDENT = mybir.ActivationFunctionType.Identity

    nc = bacc.Bacc("TRN2", target_bir_lowering=False, debug=False, num_devices=8)

    xt_d = nc.dram_tensor("xt", [TB, 4, 128, 4 * 512], bf16, kind="ExternalInput")   # x^T blocked
    wq_d = nc.dram_tensor("wq", [E, NQ], bf16, kind="ExternalInput")                 # group slice
    wkv_d = nc.dram_tensor("wkv", [E, NKV], bf16, kind="ExternalInput")              # group slice
    wo_d = nc.dram_tensor("wo", [4, 128, E], bf16, kind="ExternalInput")             # group slice
    pet_d = nc.dram_tensor("pet", [D, T], bf16, kind="ExternalInput")                # pe^T
    bq_d = nc.dram_tensor("bq", [D, QPG], f32, kind="ExternalInput")                 # col h
    bk_d = nc.dram_tensor("bk", [D, 1], f32, kind="ExternalInput")
    bv_d = nc.dram_tensor("bv", [D, 1], f32, kind="ExternalInput")
    msk_d = nc.dram_tensor("msk", [4, 128, 512], bf16, kind="ExternalInput")
    idb_d = nc.dram_tensor("idb", [128, 128], bf16, kind="ExternalInput")
    ones_d = nc.dram_tensor("ones1", [128, 1], bf16, kind="ExternalInput")
    out_d = nc.dram_tensor("out", [T, E], bf16, kind="ExternalOutput")

    with tile.TileContext(nc) as tc:
        with tc.tile_pool(name="persist", bufs=1) as pp, \
             tc.tile_pool(name="p2", bufs=28) as p2:
            qt = [pp.tile([128, T], bf16, name=f"qt{h}", tag=f"qt{h}") for h in range(QPG)]
            kt = pp.tile([128, T], bf16)
            vext = [pp.tile([128, 132], bf16, name=f"vx{i}", tag=f"vx{i}") for i in range(TT)]
            at = [pp.tile([128, T], bf16, name=f"at{h}", tag=f"at{h}") for h in range(QPG)]
            wo_sb = [pp.tile([128, E], bf16, name=f"wo{h}", tag=f"wo{h}") for h in range(QPG)]

            pet = pp.tile([D, T], bf16)
            bq = pp.tile([D, QPG], f32)
            bk = pp.tile([D, 1], f32)
            bv = pp.tile([D, 1], f32)
            idb = pp.tile([128, 128], bf16)
            ones1 = pp.tile([128, 1], bf16)
            msk = [pp.tile([128, 512], bf16, name=f"msk{j}", tag=f"msk{j}") for j in range(4)]

            pt_q0 = {}   # h -> [4 P^T tiles], scores computed inside phase 1

            # ---- phase 1: projections (+ qb0 attention scores as filler) ----
            with (
                tc.tile_pool(name="p1", bufs=1) as p1,
                tc.tile_pool(name="p1x", bufs=8) as p1x,
                tc.tile_pool(name="p1e", bufs=16) as p1e,
                tc.tile_pool(name="psA", bufs=1, space="PSUM") as psA,
                tc.tile_pool(name="ps1b", bufs=1, space="PSUM") as ps1b,
                tc.tile_pool(name="sA", bufs=1, space="PSUM") as sA,
            ):
                wq_sb = [p1.tile([128, NQ], bf16, name=f"wq{e}", tag=f"wq{e}") for e in range(NE)]
                wkv_sb = [p1.tile([128, NKV], bf16, name=f"wkv{e}", tag=f"wkv{e}") for e in range(NE)]
                xt0 = [p1e.tile([128, 512], bf16, name=f"xt0_{e}", tag="xt0") for e in range(NE)]
                # consumption-ordered fine-grained loads: scalar ring carries
                # weights, sync ring carries xt; first e-iteration's operands
                # are the first triggers on each ring.
                for e in range(NE):
                    a, b = divmod(e, 4)
                    nc.scalar.dma_start(wq_sb[e][:], wq_d[e * 128:(e + 1) * 128, :])
                    nc.scalar.dma_start(wkv_sb[e][:], wkv_d[e * 128:(e + 1) * 128, :])
                    nc.sync.dma_start(xt0[e][:], xt_d[0, a, :, b * 512:(b + 1) * 512])
                # constants on gpsimd ring, in first-use order
                nc.gpsimd.dma_start(bq[:], bq_d[:])
                nc.gpsimd.dma_start(bk[:], bk_d[:])
                nc.gpsimd.dma_start(bv[:], bv_d[:])
                nc.gpsimd.dma_start(idb[:], idb_d[:])
                nc.gpsimd.dma_start(ones1[:], ones_d[:])
                for j in range(4):
                    nc.gpsimd.dma_start(msk[j][:], msk_d[j])
                nc.gpsimd.dma_start(pet[:], pet_d[:])

                def q0_score_tile(h, tk):
                    # one qb0 score tile for head h: diagonal, suffix only
                    off = tk * 128
                    w = 512 - off
                    s_ps = sA.tile([128, 512], f32, name="s0", tag="s0")
                    nc.tensor.matmul(
                        s_ps[:, 0:w], kt[:, tk * 128:(tk + 1) * 128],
                        qt[h][:, off:512], start=True, stop=True,
                    )
                    p_t = p2.tile([128, 512], bf16, name="pt", tag="pt")
                    nc.scalar.activation(p_t[:, off:], s_ps[:, 0:w], EXP, scale=ISD)
                    nc.vector.tensor_tensor(p_t[:, off:], p_t[:, off:], msk[tk][:, off:], MULT)
                    pt_q0.setdefault(h, []).append(p_t)

                for tb in range(TB):
                    ts = slice(tb * 512, (tb + 1) * 512)
                    qt_ps = psA.tile([128, 4 * 512], f32, name="qt_ps", tag="qt_ps")
                    kt_ps = psA.tile([128, 512], f32, name="kt_ps", tag="kt_ps")
                    vt_ps = psA.tile([128, 512], f32, name="vt_ps", tag="vt_ps")
                    for e in range(NE):
                        a, b = divmod(e, 4)
                        if tb == 0:
                            xt_t = xt0[e]
                            xs = slice(0, 512)
                        else:
                            if b == 0:
                                xt_t = p1x.tile([128, 4 * 512], bf16, name="xt", tag="xt")
                                nc.sync.dma_start(xt_t[:], xt_d[tb, a])
                            xs = slice(b * 512, (b + 1) * 512)
                        st = e == 0
                        sp = e == NE - 1
                        for h in range(QPG):
                            nc.tensor.matmul(
                                qt_ps[:, h * 512:(h + 1) * 512],
                                wq_sb[e][:, h * 128:(h + 1) * 128],
                                xt_t[:, xs], start=st, stop=sp,
                            )
                        nc.tensor.matmul(kt_ps[:], wkv_sb[e][:, 0:128], xt_t[:, xs], start=st, stop=sp)
                        nc.tensor.matmul(vt_ps[:], wkv_sb[e][:, 128:256], xt_t[:, xs], start=st, stop=sp)
                        # interleave qb0 attention score tiles as DMA-stall filler
                        if tb in (1, 2) and e % 2 == 1:
                            hh, tt = divmod((tb - 1) * 8 + e // 2, 4)
                            q0_score_tile(hh, tt)
                    # fused drain: (psum + bias) + pe^T -> sbuf bf16
                    for h in range(QPG):
                        nc.vector.scalar_tensor_tensor(
                            qt[h][:, ts], qt_ps[:, h * 512:(h + 1) * 512],
                            bq[:, h:h + 1], pet[:, ts], ADD, ADD,
                        )
                    nc.vector.scalar_tensor_tensor(
                        kt[:, ts], kt_ps[:], bk[:], pet[:, ts], ADD, ADD,
                    )
                    # v: bias then cast to bf16, then transpose each 128-tile
                    vtb = p1.tile([128, 512], bf16, name="vtb", tag="vtb")
                    nc.scalar.activation(vtb[:], vt_ps[:], IDENT, bias=bv[:], scale=1.0)
                    for i in range(4):
                        ti = tb * 4 + i
                        vtp = ps1b.tile([128, 128], bf16, name="vtp", tag="vtp")
                        nc.tensor.transpose(vtp[:], vtb[:, i * 128:(i + 1) * 128], idb[:])
                        nc.vector.tensor_copy(vext[ti][:, 0:128], vtp[:])
                        nc.vector.tensor_copy(vext[ti][:, 128:129], ones1[:])

            for h in range(QPG):
                nc.gpsimd.dma_start(wo_sb[h][:], wo_d[h])

            # ---- phase 2+3: attention fused with output projection ----
            with (
                tc.tile_pool(name="p2s", bufs=8) as p2s,
                tc.tile_pool(name="p3", bufs=3) as p3,
                tc.tile_pool(name="ps2", bufs=2, space="PSUM") as ps2,
            ):
                from collections import deque
                filler = deque()

                def drain(n):
                    for _ in range(n):
                        if not filler:
                            return
                        filler.popleft()()

                def wo_units(qb):
                    units = []
                    for jj in range(4):
                        ti = qb * 4 + jj
                        state = {}

                        def alloc(state=state):
                            state["o_sb"] = p3.tile([128, E], bf16, name="osb", tag="osb")
                        units.append(alloc)
                        for eo in range(4):
                            def mmA(state=state, ti=ti, eo=eo):
                                w_ps = ps2.tile([128, 512], f32, name="w_ps", tag="mix", bufs=2)
                                state["w"] = w_ps
                                for h in range(2):
                                    nc.tensor.matmul(
                                        w_ps[:], at[h][:, ti * 128:(ti + 1) * 128],
                                        wo_sb[h][:, eo * 512:(eo + 1) * 512],
                                        start=(h == 0), stop=False,
                                    )

                            def mmB(state=state, ti=ti, eo=eo):
                                w_ps = state["w"]
                                for h in range(2, 4):
                                    nc.tensor.matmul(
                                        w_ps[:], at[h][:, ti * 128:(ti + 1) * 128],
                                        wo_sb[h][:, eo * 512:(eo + 1) * 512],
                                        start=False, stop=(h == 3),
                                    )
                                nc.vector.tensor_copy(state["o_sb"][:, eo * 512:(eo + 1) * 512], w_ps[:])
                                nc.sync.dma_start(
                                    out_d[ti * 128:(ti + 1) * 128, eo * 512:(eo + 1) * 512],
                                    state["o_sb"][:, eo * 512:(eo + 1) * 512],
                                )
                            units.append(mmA)
                            units.append(mmB)
                    return units

                def make_pv_streak(h, qb, pt, and_then=None):
                    def emit():
                        # pure bf16 PV streak: all four tq sub-tiles back to back
                        o_list = []
                        for j in range(4):
                            tt = 4 * qb + j
                            o_ps = ps2.tile([128, 129], f32, name="o_ps", tag="o_ps", bufs=4)
                            o_list.append(o_ps)
                            for tk in range(tt + 1):
                                nc.tensor.matmul(
                                    o_ps[:], pt[tk][:, j * 128:(j + 1) * 128],
                                    vext[tk][:, 0:129],
                                    start=(tk == 0), stop=(tk == tt),
                                )
                        for j in range(4):
                            tt = 4 * qb + j
                            o_ps = o_list[j]
                            r_sb = p2s.tile([128, 1], f32, name="r", tag="r")
                            nc.vector.reciprocal(r_sb[:], o_ps[:, 128:129])
                            a_sb = p2s.tile([128, 128], bf16, name="a", tag="a")
                            nc.vector.tensor_tensor(
                                a_sb[:], o_ps[:, 0:128], r_sb[:].to_broadcast([128, 128]), MULT,
                            )
                            at_ps = ps2.tile([128, 128], bf16, name="at_ps", tag="mix", bufs=2)
                            nc.tensor.transpose(at_ps[:], a_sb[:], idb[:])
                            nc.vector.tensor_copy(at[h][:, tt * 128:(tt + 1) * 128], at_ps[:])
                        if and_then is not None:
                            and_then()
                    return emit

                pv_queue = deque()

                def make_cb(qb):
                    def cb():
                        filler.extend(wo_units(qb))
                    return cb

                # qb0's P^T tiles were computed inside phase 1; its PV streaks
                # fire inside qb1-h0's score stretch so their DVE epilogues
                # overlap score matmuls instead of gating the PE.
                for h in range(QPG):
                    pv_queue.append(make_pv_streak(
                        h, 0, pt_q0[h], and_then=make_cb(0) if h == QPG - 1 else None))

                for qb in range(1, TB):
                    nkt = 4 * qb + 4
                    for h in range(QPG):
                        pt = []
                        for tk in range(nkt):
                            j = tk - 4 * qb
                            off = max(j, 0) * 128
                            w = 512 - off
                            qcols = slice(qb * 512 + off, (qb + 1) * 512)
                            s_ps = ps2.tile([128, 512], f32, name="s_ps", tag="s_ps")
                            nc.tensor.matmul(
                                s_ps[:, 0:w], kt[:, tk * 128:(tk + 1) * 128], qt[h][:, qcols],
                                start=True, stop=True,
                            )
                            p_t = p2.tile([128, 512], bf16, name="pt", tag="pt")
                            nc.scalar.activation(p_t[:, off:], s_ps[:, 0:w], EXP, scale=ISD)
                            if j >= 0:
                                nc.vector.tensor_tensor(p_t[:, off:], p_t[:, off:], msk[j][:, off:], MULT)
                            pt.append(p_t)
                            if tk % 2 == 1 and pv_queue:
                                pv_queue.popleft()()
                            else:
                                drain(1)
                        pv_queue.append(make_pv_streak(
                            h, qb, pt, and_then=make_cb(qb) if h == QPG - 1 else None))
                while pv_queue:
                    pv_queue.popleft()()
                drain(len(filler) + 1)

    nc.compile()
    return nc


def _get_compiled():
    global _compiled
    if _compiled is None:
        _compiled = _build()
    return _compiled


def _host_inputs(x, wq, bq, wkv, bkv, wo):
    import ml_dtypes

    bf = ml_dtypes.bfloat16

    pos = np.arange(T, dtype=np.float32)[:, None]
    i = np.arange(0, D, 2, dtype=np.float32)
    inv = np.exp(-(np.log(10000.0) * i / D))
    ang = pos * inv
    pe = np.zeros((T, D), np.float32)
    pe[:, 0::2] = np.sin(ang)
    pe[:, 1::2] = np.cos(ang)
    pet = np.ascontiguousarray(pe.T).astype(bf)

    # causal masks for the 4 diagonal tiles of a 512-wide tq block:
    # mask_j[p, c] = 1 if c >= 128*j + p
    c = np.arange(512)[None, :]
    p = np.arange(128)[:, None]
    msk = np.stack([(c >= 128 * j + p) for j in range(4)]).astype(bf)

    idb = np.eye(128, dtype=bf)
    ones1 = np.ones((128, 1), dtype=bf)

    # x^T blocked: [tb, a, p, b*512+c] = xT[(4a+b)*128+p, tb*512+c]
    xts = []
    for b_ in range(B):
        xT = np.ascontiguousarray(x[b_].T).astype(bf)          # [E, T]
        xb = xT.reshape(4, 4, 128, 4, 512).transpose(3, 0, 2, 1, 4).reshape(TB, 4, 128, 4 * 512)
        xts.append(np.ascontiguousarray(xb))

    in_maps = []
    for core in range(8):
        b_, g = divmod(core, G)
        wog = wo[g * NQ:(g + 1) * NQ, :].astype(bf).reshape(4, 128, E)
        in_maps.append({
            "xt": xts[b_],
            "wq": np.ascontiguousarray(wq[:, g * NQ:(g + 1) * NQ].astype(bf)),
            "wkv": np.ascontiguousarray(wkv[:, g * NKV:(g + 1) * NKV].astype(bf)),
            "wo": np.ascontiguousarray(wog),
            "pet": pet,
            "bq": np.ascontiguousarray(bq[g * NQ:(g + 1) * NQ].reshape(QPG, D).T),
            "bk": np.ascontiguousarray(bkv[g * NKV:g * NKV + D].reshape(D, 1)),
            "bv": np.ascontiguousarray(bkv[g * NKV + D:(g + 1) * NKV].reshape(D, 1)),
            "msk": msk,
            "idb": idb,
            "ones1": ones1,
        })
    return in_maps


def run(x, wq, bq, wkv, bkv, wo, trace=False):
    from concourse.bass_utils import run_bass_kernel_spmd

    nc = _get_compiled()
    in_maps = _host_inputs(
        np.asarray(x, np.float32), np.asarray(wq, np.float32),
        np.asarray(bq, np.float32), np.asarray(wkv, np.float32),
        np.asarray(bkv, np.float32), np.asarray(wo, np.float32),
    )
    res = run_bass_kernel_spmd(nc, in_maps, core_ids=list(range(8)), trace=trace)
    out = np.zeros((B, T, E), np.float32)
    for core in range(8):
        b_ = core // G
        out[b_] += res.results[core]["out"].astype(np.float32)
    return out, res


def kernel(x, wq, bq, wkv, bkv, wo):
    out, _ = run(x, wq, bq, wkv, bkv, wo, trace=False)
    return out
